# revision 5
# baseline (speedup 1.0000x reference)
"""DeepSeek-style MoE block (block-quantized SwiGLU experts, top-4 routing)
as a Bass/Tile kernel on 8 Trainium2 NeuronCores.

Strategy (expert-parallel, tokens-stationary):
  - 16 experts sharded 2-per-core; host pairs the 8 heaviest-loaded experts
    (slot A) with the 8 lightest (slot B) so per-expert token counts fit the
    chunk structure: A = 128 + CV(=32) overflow tokens, B = 128 tokens.
  - Orientation: the gathered token block is the PE's STATIONARY operand and
    the expert weights are the MOVING operand (N=512 per matmul), so weight
    reloads (LDWEIGHTS) are small and fully hidden under matmul streaming.
      stage 1:  [g|u][t, i] += x_hb[t,:]-loaded @ (w0|w1)[hb, :]    (hb: 16)
      silu:     h = sigmoid(g)*g*u                    (ACT + DVE, from PSUM)
      transp.:  hT[i, t] = PE-transpose(h)            (needed: stage-2 K = i)
      stage 2:  y[t, h] += hT_ib-loaded @ w2[ib, :]   (ib: 8)
      scale:    out = y * rw[t]     (ACT Copy with per-partition scale)
  - Weights are dequantized (128x128 block scales folded) + cast to bf16 and
    retiled on the host; w0/w1 are interleaved per h-block into one moving
    stream. Weight chunks alternate between the two HWDGE queues (sync/
    scalar) so transfers overlap; x/rw/identity and outputs ride the gpsimd
    (SWDGE) queue.

Layouts (host-prepared, per core):
  wg   [128, HB, 2048]  wg[p, hb, i]      = w0d[e, i, hb*128+p]
                        wg[p, hb, 1024+i] = w1d[e, i, hb*128+p]
  w2m  [128, IB, 2048]  w2m[p, ib, h]     = w2d[e, h, ib*128+p]
  x_t  [128, HB, C]     x_t[p, hb, t]     = x[tok_t, hb*128+p]      (bf16)
  rw_t [128, 3]         per-chunk routing weights, token on partition
  y_t  [3, 2, 128, 1024] bf16 output per (chunk, h-half)
"""

import numpy as np
import ml_dtypes

T, H, I, E, K = 512, 2048, 1024, 16, 4
BS = 128
NCORES = 8
EPC = E // NCORES      # experts per core
HB = H // 128          # 16 h-blocks
IB = I // 128          # 8 i-blocks
CV = 32                # overflow-chunk token capacity (slot A)

WG_CHUNKS = [2, 2, 4, 4, 4]   # hb per wg DMA chunk
W2_CHUNKS = [2, 2, 4]         # ib per w2 DMA chunk

WDT_NP = ml_dtypes.bfloat16

_compiled = {}         # key -> nc
_prep_w_cache = {}     # key -> (wg, w2m) per-expert arrays
LAST_RESULTS = None    # BassKernelResults of the most recent run
TRACE = False
TRACE_CORES = None


def _build(cv=CV):
    import concourse.bass as bass
    import concourse.mybir as mybir
    import concourse.tile as tile
    from concourse import bacc

    f32 = mybir.dt.float32
    wdt = mybir.dt.bfloat16
    sig = mybir.ActivationFunctionType.Sigmoid
    cpy = mybir.ActivationFunctionType.Copy

    nc = bacc.Bacc(
        "TRN2",
        target_bir_lowering=False,
        debug=False,
        enable_asserts=False,
        num_devices=NCORES,
    )

    wg_t = nc.dram_tensor("wg_t", [EPC, 128, HB, 2048], wdt, kind="ExternalInput").ap()
    w2_t = nc.dram_tensor("w2_t", [EPC, 128, IB, 2048], wdt, kind="ExternalInput").ap()
    xA_t = nc.dram_tensor("xA_t", [128, HB, 128], wdt, kind="ExternalInput").ap()
    xV_t = nc.dram_tensor("xV_t", [128, HB, cv], wdt, kind="ExternalInput").ap()
    xB_t = nc.dram_tensor("xB_t", [128, HB, 128], wdt, kind="ExternalInput").ap()
    rw_t = nc.dram_tensor("rw_t", [128, 3], f32, kind="ExternalInput").ap()
    id_t = nc.dram_tensor("id_t", [128, 128], wdt, kind="ExternalInput").ap()
    y_t = nc.dram_tensor("y_t", [3, 2, 128, 1024], wdt, kind="ExternalOutput").ap()

    with tile.TileContext(nc) as tc:
        with (
            tc.tile_pool(name="xp", bufs=1) as xp,
            tc.tile_pool(name="wgp", bufs=1) as wgp,
            tc.tile_pool(name="w2p", bufs=1) as w2p,
            tc.tile_pool(name="hp", bufs=2) as hp,
            tc.tile_pool(name="htp", bufs=2) as htp,
            tc.tile_pool(name="scp", bufs=2) as scp,
            tc.tile_pool(name="op", bufs=2) as op,
            tc.tile_pool(name="ps", bufs=4, space="PSUM") as ps,
        ):
            # --- persistent small inputs (gpsimd SWDGE queue) ---
            x_sbs = {}
            for nm, t_ap, cap in (("A", xA_t, 128), ("V", xV_t, cv), ("B", xB_t, 128)):
                x_sb = xp.tile([128, HB, cap], wdt, tag=f"x{nm}", name=f"x{nm}")
                nc.gpsimd.dma_start(x_sb[:], t_ap)
                x_sbs[nm] = x_sb
            idn = xp.tile([128, 128], wdt, tag="idn")
            nc.gpsimd.dma_start(idn[:], id_t)
            rw_sb = xp.tile([128, 3], f32, tag="rw")
            nc.gpsimd.dma_start(rw_sb[:], rw_t)

            # --- weight streams: chunks alternate across the 2 HWDGE rings ---
            wq = [nc.sync, nc.scalar]
            qi = [0]

            def wdma(dst, src):
                wq[qi[0] % 2].dma_start(dst, src)
                qi[0] += 1

            wg_sb = {}   # (e, hb) -> AP [128, 2048]
            w2_sb = {}   # (e, ib) -> AP [128, 2048]

            def load_wg(e):
                c0 = 0
                for ch in WG_CHUNKS:
                    t = wgp.tile([128, ch, 2048], wdt, tag=f"wg{ch}", bufs=2 if ch == 2 else 3,
                                 name=f"wg{e}c{c0}")
                    wdma(t[:], wg_t[e, :, c0:c0 + ch])
                    for j in range(ch):
                        wg_sb[(e, c0 + j)] = t[:, j]
                    c0 += ch

            def load_w2(e):
                c0 = 0
                for ch in W2_CHUNKS:
                    t = w2p.tile([128, ch, 2048], wdt, tag=f"w2_{e}_{c0}", bufs=1,
                                 name=f"w2{e}c{c0}")
                    wdma(t[:], w2_t[e, :, c0:c0 + ch])
                    for j in range(ch):
                        w2_sb[(e, c0 + j)] = t[:, j]
                    c0 += ch

            load_wg(0)
            load_w2(0)
            load_wg(1)
            load_w2(1)

            def stage1(chunks):
                """chunks: list of (name, cap, expert)."""
                e = chunks[0][2]
                gu = {}
                for nm, cap, _ in chunks:
                    g_ps = ps.tile([128, 1024], f32, tag="ps", name=f"g{nm}")
                    u_ps = ps.tile([128, 1024], f32, tag="ps", name=f"u{nm}")
                    gu[nm] = (g_ps, u_ps)
                for hb in range(HB):
                    w = wg_sb[(e, hb)]
                    for nm, cap, _ in chunks:
                        x_st = x_sbs[nm][:, hb]
                        g_ps, u_ps = gu[nm]
                        st, sp = (hb == 0), (hb == HB - 1)
                        nc.tensor.matmul(g_ps[:cap, 0:512], x_st, w[:, 0:512], start=st, stop=sp)
                        nc.tensor.matmul(g_ps[:cap, 512:1024], x_st, w[:, 512:1024], start=st, stop=sp)
                        nc.tensor.matmul(u_ps[:cap, 0:512], x_st, w[:, 1024:1536], start=st, stop=sp)
                        nc.tensor.matmul(u_ps[:cap, 512:1024], x_st, w[:, 1536:2048], start=st, stop=sp)
                hs = {}
                for nm, cap, _ in chunks:
                    g_ps, u_ps = gu[nm]
                    sg = scp.tile([128, 1024], f32, tag="sc", name=f"sg{nm}")
                    nc.scalar.activation(sg[:cap], g_ps[:cap], sig)
                    p1 = scp.tile([128, 1024], f32, tag="sc", name=f"p1{nm}")
                    nc.vector.tensor_mul(p1[:cap], sg[:cap], g_ps[:cap])
                    h_sb = hp.tile([128, 1024], wdt, tag="h", name=f"h{nm}")
                    nc.vector.tensor_mul(h_sb[:cap], p1[:cap], u_ps[:cap])
                    hs[nm] = h_sb
                return hs

            def transpose_h(nm, cap, h_sb):
                hT = htp.tile([128, IB, cap], wdt, tag="ht", name=f"hT{nm}")
                for ib in range(IB):
                    tr = ps.tile([128, cap], wdt, tag="ps", name=f"tr{nm}{ib}")
                    nc.tensor.transpose(tr[:], h_sb[:cap, ib * 128:(ib + 1) * 128],
                                        idn[:cap, :cap])
                    nc.vector.tensor_copy(hT[:, ib], tr[:])
                return hT

            def stage2(nm, cap, e, ci, hT):
                o_sb = op.tile([128, 2048], wdt, tag="o", name=f"o{nm}")
                y_ps = [ps.tile([128, 1024], f32, tag="ps", name=f"y{nm}{yh}")
                        for yh in range(2)]
                for ib in range(IB):
                    w = w2_sb[(e, ib)]
                    st, sp = (ib == 0), (ib == IB - 1)
                    for yh in range(2):
                        o0 = yh * 1024
                        nc.tensor.matmul(y_ps[yh][:cap, 0:512], hT[:, ib],
                                         w[:, o0:o0 + 512], start=st, stop=sp)
                        nc.tensor.matmul(y_ps[yh][:cap, 512:1024], hT[:, ib],
                                         w[:, o0 + 512:o0 + 1024], start=st, stop=sp)
                for yh in range(2):
                    o0 = yh * 1024
                    nc.scalar.activation(o_sb[:cap, o0:o0 + 1024], y_ps[yh][:cap],
                                         cpy, scale=rw_sb[:cap, ci:ci + 1])
                    nc.gpsimd.dma_start(y_t[ci, yh, :cap], o_sb[:cap, o0:o0 + 1024])

            # ---- schedule ----
            hsA = stage1([("A", 128, 0), ("V", cv, 0)])
            hTA = transpose_h("A", 128, hsA["A"])
            hTV = transpose_h("V", cv, hsA["V"])
            stage2("A", 128, 0, 0, hTA)
            stage2("V", cv, 0, 1, hTV)
            hsB = stage1([("B", 128, 1)])
            hTB = transpose_h("B", 128, hsB["B"])
            stage2("B", 128, 1, 2, hTB)

    nc.compile()
    return nc


def _route(selected_experts):
    se = np.asarray(selected_experts).astype(np.int64).ravel()  # [T*K]
    order = np.argsort(se, kind="stable")                       # slots by expert
    counts = np.bincount(se, minlength=E)
    starts = np.zeros(E + 1, dtype=np.int64)
    np.cumsum(counts, out=starts[1:])
    return order, counts, starts


def _prep_weights(w0, w1, w2, s0, s1, s2):
    """Dequantize (fold 128x128 block scales), retile, cast bf16, interleave."""
    w0 = np.asarray(w0, dtype=np.float32)
    w1 = np.asarray(w1, dtype=np.float32)
    w2 = np.asarray(w2, dtype=np.float32)
    s0 = np.asarray(s0, dtype=np.float32)
    s1 = np.asarray(s1, dtype=np.float32)
    s2 = np.asarray(s2, dtype=np.float32)
    w0b = w0.reshape(E, IB, BS, HB, BS) * s0[:, :, None, :, None]
    w1b = w1.reshape(E, IB, BS, HB, BS) * s1[:, :, None, :, None]
    w2b = w2.reshape(E, HB, BS, IB, BS) * s2[:, :, None, :, None]
    w0d = w0b.reshape(E, I, H)
    w1d = w1b.reshape(E, I, H)
    w2d = w2b.reshape(E, H, I)
    # wg[e, p, hb, 0:1024]=w0d[e, :, hb*128+p]; [1024:2048]=w1d
    w0t = w0d.reshape(E, I, HB, BS).transpose(0, 3, 2, 1)   # [E,128,HB,I]
    w1t = w1d.reshape(E, I, HB, BS).transpose(0, 3, 2, 1)
    wg = np.ascontiguousarray(
        np.concatenate([w0t, w1t], axis=3).astype(WDT_NP))  # [E,128,HB,2048]
    # w2m[e, p, ib, h] = w2d[e, h, ib*128+p]
    w2m = np.ascontiguousarray(
        w2d.reshape(E, H, IB, BS).transpose(0, 3, 2, 1).astype(WDT_NP))
    return wg, w2m


def _gather_x(x16, toks, cap):
    """x16 [T, H] bf16 -> [128, HB, cap] stationary tile."""
    n = len(toks)
    out = np.zeros((128, HB, cap), dtype=WDT_NP)
    if n:
        xe = x16[toks]                                   # [n, H]
        out[:, :, :n] = xe.T.reshape(HB, BS, n).transpose(1, 0, 2)
    return out


def kernel(x, w0, w1, w2, s0, s1, s2, selected_experts, routing_weights):
    global LAST_RESULTS
    from concourse.bass_utils import run_bass_kernel_spmd

    x = np.asarray(x, dtype=np.float32)
    routing_weights = np.asarray(routing_weights, dtype=np.float32)

    order, counts, starts = _route(selected_experts)
    eorder = np.argsort(-counts, kind="stable")
    heavy = eorder[:NCORES]
    light = eorder[NCORES:][::-1]        # lightest paired with heaviest
    assert counts[heavy].max() <= 128 + CV, counts
    assert counts[light].max() <= 128, counts

    wkey = (id(w0), id(w1), id(w2), id(s0), id(s1), id(s2))
    if wkey not in _prep_w_cache:
        _prep_w_cache.clear()
        _prep_w_cache[wkey] = _prep_weights(w0, w1, w2, s0, s1, s2)
    wg, w2m = _prep_w_cache[wkey]

    if CV not in _compiled:
        _compiled[CV] = _build(CV)
    nc = _compiled[CV]

    x16 = x.astype(WDT_NP)
    rw_flat = routing_weights.ravel()
    tok_of_slot = order // K
    idn = np.eye(128, dtype=WDT_NP)

    in_maps = []
    core_slots = []
    for m in range(NCORES):
        eA, eB = int(heavy[m]), int(light[m])
        slA = order[starts[eA]:starts[eA] + counts[eA]]
        slB = order[starts[eB]:starts[eB] + counts[eB]]
        chunks = [slA[:128], slA[128:], slB]
        core_slots.append(chunks)
        rw_core = np.zeros((128, 3), dtype=np.float32)
        for c, sl in enumerate(chunks):
            rw_core[:len(sl), c] = rw_flat[sl]
        in_maps.append({
            "wg_t": wg[[eA, eB]],
            "w2_t": w2m[[eA, eB]],
            "xA_t": _gather_x(x16, tok_of_slot[starts[eA]:starts[eA] + len(chunks[0])], 128),
            "xV_t": _gather_x(x16, tok_of_slot[starts[eA] + 128:starts[eA] + counts[eA]], CV),
            "xB_t": _gather_x(x16, tok_of_slot[starts[eB]:starts[eB] + counts[eB]], 128),
            "rw_t": rw_core,
            "id_t": idn,
        })

    res = run_bass_kernel_spmd(
        nc, in_maps, core_ids=list(range(NCORES)),
        trace=TRACE, trace_cores=TRACE_CORES)
    LAST_RESULTS = res

    out = np.zeros((T * K, H), dtype=np.float32)
    for m in range(NCORES):
        y_core = res.results[m]["y_t"]   # [3, 2, 128, 1024] bf16
        for c, sl in enumerate(core_slots[m]):
            n = len(sl)
            if n:
                out[sl, 0:1024] = y_core[c, 0, :n].astype(np.float32)
                out[sl, 1024:2048] = y_core[c, 1, :n].astype(np.float32)
    return out.reshape(T, K, H)


# revision 14
# speedup vs baseline: 1.0435x; 1.0435x over previous
"""DeepSeek-style MoE block (block-quantized SwiGLU experts, top-4 routing)
as a Bass/Tile kernel on 8 Trainium2 NeuronCores.

Strategy (expert-parallel, tokens-stationary):
  - 16 experts sharded 2-per-core; host pairs the 8 heaviest-loaded experts
    (slot A) with the 8 lightest (slot B) so per-expert token counts fit the
    chunk structure: A = 128 + CV(=32) overflow tokens, B = 128 tokens.
  - Orientation: the gathered token block is the PE's STATIONARY operand and
    the expert weights are the MOVING operand (N=512 per matmul), so weight
    reloads (LDWEIGHTS) are small and fully hidden under matmul streaming.
      stage 1:  [g|u][t, i] += x_hb[t,:]-loaded @ (w0|w1)[hb, :]    (hb: 16)
      silu:     h = sigmoid(g)*g*u                    (ACT + DVE, from PSUM)
      transp.:  hT[i, t] = PE-transpose(h)            (needed: stage-2 K = i)
      stage 2:  y[t, h] += hT_ib-loaded @ w2[ib, :]   (ib: 8)
      scale:    out = y * rw[t]     (ACT Copy with per-partition scale)
  - Weights are dequantized (128x128 block scales folded) + cast to bf16 and
    retiled on the host; w0/w1 are interleaved per h-block into one moving
    stream. Weight chunks alternate between the two HWDGE queues (sync/
    scalar) so transfers overlap; x/rw/identity and outputs ride the gpsimd
    (SWDGE) queue.

Layouts (host-prepared, per core):
  wg   [128, HB, 2048]  wg[p, hb, i]      = w0d[e, i, hb*128+p]
                        wg[p, hb, 1024+i] = w1d[e, i, hb*128+p]
  w2m  [128, IB, 2048]  w2m[p, ib, h]     = w2d[e, h, ib*128+p]
  x_t  [128, HB, C]     x_t[p, hb, t]     = x[tok_t, hb*128+p]      (bf16)
  rw_t [128, 3]         per-chunk routing weights, token on partition
  y_t  [3, 2, 128, 1024] bf16 output per (chunk, h-half)
"""

import numpy as np
import ml_dtypes

T, H, I, E, K = 512, 2048, 1024, 16, 4
BS = 128
NCORES = 8
EPC = E // NCORES      # experts per core
HB = H // 128          # 16 h-blocks
IB = I // 128          # 8 i-blocks
CV = 32                # overflow-chunk token capacity (slot A)

WG_CHUNKS = [1, 3, 4, 4, 4]   # hb per wg DMA chunk (first small: compute starts early)
W2_CHUNKS = [2, 2, 4]         # ib per w2 DMA chunk
N_WARM = 72                   # dummy matmuls to hold the PE HAM clock warm

WDT_NP = ml_dtypes.bfloat16

_compiled = {}         # key -> nc
_prep_w_cache = {}     # key -> (wg, w2m) per-expert arrays
LAST_RESULTS = None    # BassKernelResults of the most recent run
TRACE = False
TRACE_CORES = None


def _build(cv=CV):
    import concourse.bass as bass
    import concourse.mybir as mybir
    import concourse.tile as tile
    from concourse import bacc

    f32 = mybir.dt.float32
    wdt = mybir.dt.bfloat16
    sig = mybir.ActivationFunctionType.Sigmoid
    cpy = mybir.ActivationFunctionType.Copy

    nc = bacc.Bacc(
        "TRN2",
        target_bir_lowering=False,
        debug=False,
        enable_asserts=False,
        num_devices=NCORES,
    )

    wg_t = nc.dram_tensor("wg_t", [EPC, 128, HB, 2048], wdt, kind="ExternalInput").ap()
    w2_t = nc.dram_tensor("w2_t", [EPC, 128, IB, 2048], wdt, kind="ExternalInput").ap()
    xA_t = nc.dram_tensor("xA_t", [128, HB, 128], wdt, kind="ExternalInput").ap()
    xV_t = nc.dram_tensor("xV_t", [128, HB, cv], wdt, kind="ExternalInput").ap()
    xB_t = nc.dram_tensor("xB_t", [128, HB, 128], wdt, kind="ExternalInput").ap()
    rw_t = nc.dram_tensor("rw_t", [128, 3], f32, kind="ExternalInput").ap()
    id_t = nc.dram_tensor("id_t", [128, 128], wdt, kind="ExternalInput").ap()
    y_t = nc.dram_tensor("y_t", [3, 2, 128, 1024], wdt, kind="ExternalOutput").ap()

    with tile.TileContext(nc) as tc:
        with (
            tc.tile_pool(name="xp", bufs=1) as xp,
            tc.tile_pool(name="wgp", bufs=1) as wgp,
            tc.tile_pool(name="w2p", bufs=1) as w2p,
            tc.tile_pool(name="hp", bufs=2) as hp,
            tc.tile_pool(name="htp", bufs=2) as htp,
            tc.tile_pool(name="scp", bufs=2) as scp,
            tc.tile_pool(name="op", bufs=2) as op,
            tc.tile_pool(name="ps", bufs=4, space="PSUM") as ps,
        ):
            # --- identity/rw (needed later) go to the gpsimd SWDGE queue ---
            idn = xp.tile([128, 128], wdt, tag="idn")
            nc.gpsimd.dma_start(idn[:], id_t)
            rw_sb = xp.tile([128, 3], f32, tag="rw")
            nc.gpsimd.dma_start(rw_sb[:], rw_t)

            # --- PE warm-up: keep the HAM activity monitor at full clock
            # while the first weight chunks stream in. Results discarded. ---
            warm_sb = xp.tile([128, 128], wdt, tag="warm")
            nc.vector.memset(warm_sb[:], 0)
            warm_ps = ps.tile([128, 128], f32, tag="ps")
            for _ in range(N_WARM):
                nc.tensor.matmul(warm_ps[:], warm_sb[:], warm_sb[:],
                                 start=True, stop=True)

            # --- weight streams: chunks alternate across the 2 HWDGE rings ---
            wq = [nc.sync, nc.scalar]
            qi = [0]

            def wdma(dst, src):
                wq[qi[0] % 2].dma_start(dst, src)
                qi[0] += 1

            wg_sb = {}   # (e, hb) -> AP [128, 2048]
            w2_sb = {}   # (e, ib) -> AP [128, 2048]

            def load_wg(e, skip_first=False):
                c0 = 0
                for k, ch in enumerate(WG_CHUNKS):
                    if not (skip_first and k == 0):
                        t = wgp.tile([128, ch, 2048], wdt, tag=f"wg{ch}_{k}",
                                     bufs=1, name=f"wg{e}c{c0}")
                        wdma(t[:], wg_t[e, :, c0:c0 + ch])
                        for j in range(ch):
                            wg_sb[(e, c0 + j)] = t[:, j]
                    c0 += ch

            def load_w2(e):
                c0 = 0
                for ch in W2_CHUNKS:
                    t = w2p.tile([128, ch, 2048], wdt, tag=f"w2_{e}_{c0}", bufs=1,
                                 name=f"w2{e}c{c0}")
                    wdma(t[:], w2_t[e, :, c0:c0 + ch])
                    for j in range(ch):
                        w2_sb[(e, c0 + j)] = t[:, j]
                    c0 += ch

            # startup order on the sync ring: xA, first (tiny) wg chunk, then
            # xV/xB — the first matmul's deps land with minimal queueing.
            x_sbs = {}
            xA_sb = xp.tile([128, HB, 128], wdt, tag="xA", name="xA")
            nc.sync.dma_start(xA_sb[:], xA_t)
            x_sbs["A"] = xA_sb
            c0 = WG_CHUNKS[0]
            t = wgp.tile([128, c0, 2048], wdt, tag=f"wg{c0}_0", bufs=1, name="wg0c0")
            nc.sync.dma_start(t[:], wg_t[0, :, 0:c0])
            for j in range(c0):
                wg_sb[(0, j)] = t[:, j]
            qi[0] = 1
            for nm, t_ap, cap in (("V", xV_t, cv), ("B", xB_t, 128)):
                x_sb = xp.tile([128, HB, cap], wdt, tag=f"x{nm}", name=f"x{nm}")
                nc.sync.dma_start(x_sb[:], t_ap)
                x_sbs[nm] = x_sb

            load_wg(0, skip_first=True)
            load_w2(0)
            load_wg(1)
            load_w2(1)

            def stage1(chunks):
                """chunks: list of (name, cap, expert)."""
                e = chunks[0][2]
                gu = {}
                for nm, cap, _ in chunks:
                    g_ps = ps.tile([128, 1024], f32, tag="ps", name=f"g{nm}")
                    u_ps = ps.tile([128, 1024], f32, tag="ps", name=f"u{nm}")
                    gu[nm] = (g_ps, u_ps)
                for hb in range(HB):
                    w = wg_sb[(e, hb)]
                    for nm, cap, _ in chunks:
                        x_st = x_sbs[nm][:, hb]
                        g_ps, u_ps = gu[nm]
                        st, sp = (hb == 0), (hb == HB - 1)
                        nc.tensor.matmul(g_ps[:cap, 0:512], x_st, w[:, 0:512], start=st, stop=sp)
                        nc.tensor.matmul(g_ps[:cap, 512:1024], x_st, w[:, 512:1024], start=st, stop=sp)
                        nc.tensor.matmul(u_ps[:cap, 0:512], x_st, w[:, 1024:1536], start=st, stop=sp)
                        nc.tensor.matmul(u_ps[:cap, 512:1024], x_st, w[:, 1536:2048], start=st, stop=sp)
                hs = {}
                for nm, cap, _ in chunks:
                    g_ps, u_ps = gu[nm]
                    sg = scp.tile([128, 1024], f32, tag="sc", name=f"sg{nm}")
                    p1 = scp.tile([128, 1024], f32, tag="sc", name=f"p1{nm}")
                    h_sb = hp.tile([128, 1024], wdt, tag="h", name=f"h{nm}")
                    for i0 in (0, 512):   # halves: transposes can start earlier
                        sl = slice(i0, i0 + 512)
                        nc.scalar.activation(sg[:cap, sl], g_ps[:cap, sl], sig)
                        nc.vector.tensor_mul(p1[:cap, sl], sg[:cap, sl], g_ps[:cap, sl])
                        nc.vector.tensor_mul(h_sb[:cap, sl], p1[:cap, sl], u_ps[:cap, sl])
                    hs[nm] = h_sb
                return hs

            def transpose_h(nm, cap, h_sb):
                hT = htp.tile([128, IB, cap], wdt, tag="ht", name=f"hT{nm}")
                for ib in range(IB):
                    tr = ps.tile([128, cap], wdt, tag="ps", name=f"tr{nm}{ib}")
                    nc.tensor.transpose(tr[:], h_sb[:cap, ib * 128:(ib + 1) * 128],
                                        idn[:cap, :cap])
                    nc.vector.tensor_copy(hT[:, ib], tr[:])
                return hT

            def stage2(nm, cap, e, ci, hT):
                # yh-outer: each 1024-wide output half fully accumulates then
                # drains (ACT scale + DMA) while the next half's matmuls run.
                o_sb = op.tile([128, 2048], wdt, tag="o", name=f"o{nm}")
                for yh in range(2):
                    y_ps = ps.tile([128, 1024], f32, tag="ps", name=f"y{nm}{yh}")
                    o0 = yh * 1024
                    for ib in range(IB):
                        w = w2_sb[(e, ib)]
                        st, sp = (ib == 0), (ib == IB - 1)
                        nc.tensor.matmul(y_ps[:cap, 0:512], hT[:, ib],
                                         w[:, o0:o0 + 512], start=st, stop=sp)
                        nc.tensor.matmul(y_ps[:cap, 512:1024], hT[:, ib],
                                         w[:, o0 + 512:o0 + 1024], start=st, stop=sp)
                    nc.scalar.activation(o_sb[:cap, o0:o0 + 1024], y_ps[:cap],
                                         cpy, scale=rw_sb[:cap, ci:ci + 1])
                    nc.gpsimd.dma_start(y_t[ci, yh, :cap], o_sb[:cap, o0:o0 + 1024])

            # ---- schedule (phase order keeps the PE fed: transposes follow
            # their silu; s1B slots between s2A and s2V so PSUM drains of one
            # phase overlap matmuls of the next) ----
            hsA = stage1([("A", 128, 0), ("V", cv, 0)])
            hTA = transpose_h("A", 128, hsA["A"])
            hTV = transpose_h("V", cv, hsA["V"])
            stage2("A", 128, 0, 0, hTA)
            hsB = stage1([("B", 128, 1)])
            stage2("V", cv, 0, 1, hTV)
            hTB = transpose_h("B", 128, hsB["B"])
            stage2("B", 128, 1, 2, hTB)

    nc.compile()
    return nc


def _route(selected_experts):
    se = np.asarray(selected_experts).astype(np.int64).ravel()  # [T*K]
    order = np.argsort(se, kind="stable")                       # slots by expert
    counts = np.bincount(se, minlength=E)
    starts = np.zeros(E + 1, dtype=np.int64)
    np.cumsum(counts, out=starts[1:])
    return order, counts, starts


def _prep_weights(w0, w1, w2, s0, s1, s2):
    """Dequantize (fold 128x128 block scales), retile, cast bf16, interleave."""
    w0 = np.asarray(w0, dtype=np.float32)
    w1 = np.asarray(w1, dtype=np.float32)
    w2 = np.asarray(w2, dtype=np.float32)
    s0 = np.asarray(s0, dtype=np.float32)
    s1 = np.asarray(s1, dtype=np.float32)
    s2 = np.asarray(s2, dtype=np.float32)
    w0b = w0.reshape(E, IB, BS, HB, BS) * s0[:, :, None, :, None]
    w1b = w1.reshape(E, IB, BS, HB, BS) * s1[:, :, None, :, None]
    w2b = w2.reshape(E, HB, BS, IB, BS) * s2[:, :, None, :, None]
    w0d = w0b.reshape(E, I, H)
    w1d = w1b.reshape(E, I, H)
    w2d = w2b.reshape(E, H, I)
    # wg[e, p, hb, 0:1024]=w0d[e, :, hb*128+p]; [1024:2048]=w1d
    w0t = w0d.reshape(E, I, HB, BS).transpose(0, 3, 2, 1)   # [E,128,HB,I]
    w1t = w1d.reshape(E, I, HB, BS).transpose(0, 3, 2, 1)
    wg = np.ascontiguousarray(
        np.concatenate([w0t, w1t], axis=3).astype(WDT_NP))  # [E,128,HB,2048]
    # w2m[e, p, ib, h] = w2d[e, h, ib*128+p]
    w2m = np.ascontiguousarray(
        w2d.reshape(E, H, IB, BS).transpose(0, 3, 2, 1).astype(WDT_NP))
    return wg, w2m


def _gather_x(x16, toks, cap):
    """x16 [T, H] bf16 -> [128, HB, cap] stationary tile."""
    n = len(toks)
    out = np.zeros((128, HB, cap), dtype=WDT_NP)
    if n:
        xe = x16[toks]                                   # [n, H]
        out[:, :, :n] = xe.T.reshape(HB, BS, n).transpose(1, 0, 2)
    return out


def kernel(x, w0, w1, w2, s0, s1, s2, selected_experts, routing_weights):
    global LAST_RESULTS
    from concourse.bass_utils import run_bass_kernel_spmd

    x = np.asarray(x, dtype=np.float32)
    routing_weights = np.asarray(routing_weights, dtype=np.float32)

    order, counts, starts = _route(selected_experts)
    eorder = np.argsort(-counts, kind="stable")
    heavy = eorder[:NCORES]
    light = eorder[NCORES:][::-1]        # lightest paired with heaviest
    assert counts[heavy].max() <= 128 + CV, counts
    assert counts[light].max() <= 128, counts

    wkey = (id(w0), id(w1), id(w2), id(s0), id(s1), id(s2))
    if wkey not in _prep_w_cache:
        _prep_w_cache.clear()
        _prep_w_cache[wkey] = _prep_weights(w0, w1, w2, s0, s1, s2)
    wg, w2m = _prep_w_cache[wkey]

    if CV not in _compiled:
        _compiled[CV] = _build(CV)
    nc = _compiled[CV]

    x16 = x.astype(WDT_NP)
    rw_flat = routing_weights.ravel()
    tok_of_slot = order // K
    idn = np.eye(128, dtype=WDT_NP)

    in_maps = []
    core_slots = []
    for m in range(NCORES):
        eA, eB = int(heavy[m]), int(light[m])
        slA = order[starts[eA]:starts[eA] + counts[eA]]
        slB = order[starts[eB]:starts[eB] + counts[eB]]
        chunks = [slA[:128], slA[128:], slB]
        core_slots.append(chunks)
        rw_core = np.zeros((128, 3), dtype=np.float32)
        for c, sl in enumerate(chunks):
            rw_core[:len(sl), c] = rw_flat[sl]
        in_maps.append({
            "wg_t": wg[[eA, eB]],
            "w2_t": w2m[[eA, eB]],
            "xA_t": _gather_x(x16, tok_of_slot[starts[eA]:starts[eA] + len(chunks[0])], 128),
            "xV_t": _gather_x(x16, tok_of_slot[starts[eA] + 128:starts[eA] + counts[eA]], CV),
            "xB_t": _gather_x(x16, tok_of_slot[starts[eB]:starts[eB] + counts[eB]], 128),
            "rw_t": rw_core,
            "id_t": idn,
        })

    res = run_bass_kernel_spmd(
        nc, in_maps, core_ids=list(range(NCORES)),
        trace=TRACE, trace_cores=TRACE_CORES)
    LAST_RESULTS = res

    out = np.zeros((T * K, H), dtype=np.float32)
    for m in range(NCORES):
        y_core = res.results[m]["y_t"]   # [3, 2, 128, 1024] bf16
        for c, sl in enumerate(core_slots[m]):
            n = len(sl)
            if n:
                out[sl, 0:1024] = y_core[c, 0, :n].astype(np.float32)
                out[sl, 1024:2048] = y_core[c, 1, :n].astype(np.float32)
    return out.reshape(T, K, H)
